# revision 1
# baseline (speedup 1.0000x reference)
"""Distributed causal multi-head attention for Trainium2 (8 NeuronCores).

Problem (hardcoded): x[2, 2048, 1024], 16 heads, head_dim 64, causal
softmax(QK^T/8)V then out-proj with bias. f32 in/out.

Sharding: data parallel on batch (cores 0-3 -> batch 0, 4-7 -> batch 1),
tensor parallel on heads within each group of 4 (4 heads per core).
Each core:
  - computes Q^T,K^T (head pairs packed to 128 partitions), V for its 4 heads
  - scores transposed S^T[k,q] = K Q^T so the softmax denominator comes out
    of the PE via an appended ones-column on V (no partition reductions)
  - exp without max-subtraction (scores are O(2), safe in fp32/bf16)
  - causal mask applied post-exp as a 0/1 bf16 multiply (DVE 4x mode)
  - ctx^T accumulated per q-chunk, normalized with 1/den partition-broadcast
  - AllGather of ctx^T bf16 [256,2048] within the 4-core group
  - column-parallel out-proj: outT[oc,q] = Wo[:,oc]^T ctxT + bo[oc]
Host assembles out[b, :, oc_slice] from each core's outT.

All matmuls bf16 (fp32 PSUM accumulation): measured end-to-end rel err
(Frobenius) ~3e-3 vs the f32 reference.
"""

import numpy as np
import ml_dtypes

from concourse import bass, bacc, mybir
from concourse import tile
from concourse.bass_utils import run_bass_kernel_spmd

BF16 = mybir.dt.bfloat16
F32 = mybir.dt.float32
Act = mybir.ActivationFunctionType

B, S, D = 2, 2048, 1024
H, HD = 16, 64
NCORES = 8
GROUP = 4            # cores per batch group
HPC = H // GROUP     # 4 heads per core
CW = HPC * HD        # 256 columns per core
QC = 512             # q-chunk width
KC = 128             # k-chunk width
NQ = S // QC         # 4
NKC = S // KC        # 16
KPQ = QC // KC       # 4 k-chunks per q-chunk
DCH = D // 128       # 8 contraction chunks of 128

_CACHE = {}


def _build_bass(reps=1):
    nc = bacc.Bacc(
        "TRN2", target_bir_lowering=False, debug=False, num_devices=NCORES
    )
    # Tile under-syncs readers of async collective outputs (readback DMAs can
    # fire before the gather lands); completion waits are attached post-Tile
    _ccs = []
    _rds = []

    # per-core external inputs (same shapes on every core: SPMD)
    xT = nc.declare_dram_parameter("xT", [D, S], BF16, isOutput=False)
    wq = nc.declare_dram_parameter("wq", [D, CW], BF16, isOutput=False)
    wk = nc.declare_dram_parameter("wk", [D, CW], BF16, isOutput=False)
    wv = nc.declare_dram_parameter("wv", [D, CW], BF16, isOutput=False)
    wo = nc.declare_dram_parameter("wo", [D, CW], BF16, isOutput=False)
    bo = nc.declare_dram_parameter("bo", [CW, 1], F32, isOutput=False)
    msk = nc.declare_dram_parameter("msk", [128, KPQ, QC], BF16, isOutput=False)
    vones = nc.declare_dram_parameter("vones", [128, NKC, HPC, 1], BF16, isOutput=False)
    # selector for den broadcast: bc[m,q] = sum_k sel33[k,m]*den_pair[k,q]
    sel33 = nc.declare_dram_parameter("sel33", [33, 128], BF16, isOutput=False)
    outT = nc.declare_dram_parameter("outT", [CW, S], F32, isOutput=True)

    with tile.TileContext(nc) as tc:
        with tc.tile_pool(name="dram", bufs=1, space="DRAM") as dram:
            # one gather per head-pair so comm overlaps the next pair's
            # attention. Shared addr_space needs >4-core groups; Local here.
            cc_in = [dram.tile([128, S], BF16, name=f"cc_in{p}") for p in range(2)]
            cc_out = [dram.tile([GROUP * 128, S], BF16, name=f"cc_out{p}")
                      for p in range(2)]

            with tc.tile_pool(name="persist", bufs=1) as pp:
                # lives across the whole kernel: ~92 KB/partition
                wq_sb = pp.tile([128, DCH, CW], BF16, tag="wq_sb")
                wk_sb = pp.tile([128, DCH, CW], BF16, tag="wk_sb")
                wv_sb = pp.tile([128, DCH, CW], BF16, tag="wv_sb")
                wo_sb = pp.tile([128, DCH, CW], BF16, tag="wo_sb")
                bo_sb = pp.tile([128, CW // 128, 1], F32, tag="bo_sb")
                msk_sb = pp.tile([128, KPQ, QC], BF16, tag="msk_sb")
                qT_sb = pp.tile([128, 2, S], BF16, tag="qT_sb")
                kT_sb = pp.tile([128, 2, S], BF16, tag="kT_sb")
                v_aug = pp.tile([128, NKC, HPC, HD + 1], BF16, tag="v_aug")
                ctxu0 = pp.tile([128, S], F32, tag="ctxu0")
                ctxu1 = pp.tile([128, S], F32, tag="ctxu1")
                # den per pair: head 2p at partition 0, head 2p+1 at partition
                # 32 (ACT writes must start at multiples of 32); rows 1-31 are
                # zeroed so the K=33 selector matmul can broadcast both heads
                # to output partitions 0-63 / 64-127 in one instruction
                den_pair = [pp.tile([33, S], BF16, tag=f"den{p}", name=f"den{p}")
                            for p in range(2)]
                sel_sb = pp.tile([33, 128], BF16, tag="sel_sb")
                ctxu_pair = [ctxu0, ctxu1]
                for p in range(2):
                    nc.vector.memset(den_pair[p][:], 0.0)

                # DMA order matters for startup latency: x first so the
                # projection matmuls can start streaming, wo/bo last
                xT_sb = pp.tile([128, DCH, S], BF16, tag="xT_sb")
                for c in range(DCH):
                    nc.sync.dma_start(xT_sb[:, c, :], xT[c * 128:(c + 1) * 128, :])
                for w_sb, w in ((wq_sb, wq), (wk_sb, wk), (wv_sb, wv)):
                    for c in range(DCH):
                        nc.sync.dma_start(w_sb[:, c, :], w[c * 128:(c + 1) * 128, :])
                nc.sync.dma_start(msk_sb[:], msk[:])
                # ones column of V_aug comes from the host: keeps the V
                # PSUM->SBUF copy to a single (PE) sync wait
                nc.sync.dma_start(v_aug[:, :, :, HD:HD + 1], vones[:])
                nc.sync.dma_start(sel_sb[:], sel33[:])
                for c in range(DCH):
                    nc.sync.dma_start(wo_sb[:, c, :], wo[c * 128:(c + 1) * 128, :])
                for o in range(CW // 128):
                    nc.sync.dma_start(bo_sb[:, o, :], bo[o * 128:(o + 1) * 128, :])

              # reps>1 repeats the whole computation for differential
              # wall-clock timing (no NTFF profiling path in this setup)
                def _emit_once():
                    # All PSUM pools coexist (phases interleave): 2+4+2 banks
                    with tc.tile_pool(name="proj_ps", bufs=2, space="PSUM") as projp, \
                         tc.tile_pool(name="sc_ps", bufs=2, space="PSUM") as scp, \
                         tc.tile_pool(name="ctbc_ps", bufs=2, space="PSUM") as ctp, \
                         tc.tile_pool(name="es_pool", bufs=NKC // 2 + 2) as esp, \
                         tc.tile_pool(name="norm", bufs=2) as np_pool:

                        def proj_qk(pair):
                            for w_sb, dst in ((wq_sb, qT_sb), (wk_sb, kT_sb)):
                                for j in range(NQ):
                                    ps = projp.tile([128, QC], F32, tag="proj")
                                    for c in range(DCH):
                                        nc.tensor.matmul(
                                            ps[:],
                                            w_sb[:, c, pair * 128:(pair + 1) * 128],
                                            xT_sb[:, c, j * QC:(j + 1) * QC],
                                            start=(c == 0),
                                            stop=(c == DCH - 1),
                                        )
                                    nc.vector.tensor_copy(
                                        dst[:, pair, j * QC:(j + 1) * QC], ps[:]
                                    )

                        def proj_v(pair):
                            # V for this pair's 2 heads: [tok, 2*64]
                            for t in range(NKC):
                                ps = projp.tile([128, QC], F32, tag="proj")
                                for c in range(DCH):
                                    nc.tensor.matmul(
                                        ps[:, 0:128],
                                        xT_sb[:, c, t * 128:(t + 1) * 128],
                                        wv_sb[:, c, pair * 128:(pair + 1) * 128],
                                        start=(c == 0),
                                        stop=(c == DCH - 1),
                                    )
                                nc.vector.tensor_copy(
                                    v_aug[:, t, 2 * pair:2 * pair + 2, 0:HD],
                                    ps[:, 0:128].rearrange("p (h w) -> p h w", h=2),
                                )

                        def attn_head(h):
                            pair, hh = h // 2, h % 2
                            row = hh * 64
                            for j in range(NQ):
                                nkc = (j + 1) * KPQ
                                qs = slice(j * QC, (j + 1) * QC)
                                es_tiles = []
                                for c0 in range(0, nkc, 2):
                                    # two k-chunks share one 2-bank PSUM tile
                                    # -> one exp instruction
                                    st = scp.tile([128, 2, QC], F32, tag="st")
                                    for i in range(2):
                                        c = c0 + i
                                        nc.tensor.matmul(
                                            st[:, i, :],
                                            kT_sb[row:row + 64, pair, c * KC:(c + 1) * KC],
                                            qT_sb[row:row + 64, pair, qs],
                                            start=True, stop=True,
                                        )
                                    es = esp.tile([128, 2, QC], BF16, tag="es")
                                    nc.scalar.activation(es[:], st[:], Act.Exp, scale=0.125)
                                    if c0 >= j * KPQ:
                                        r = c0 - j * KPQ
                                        nc.vector.tensor_mul(
                                            es[:], es[:], msk_sb[:, r:r + 2, :]
                                        )
                                    es_tiles.append(es)
                                ct = ctp.tile([HD + 1, QC], F32, tag="ct")
                                for c in range(nkc):
                                    nc.tensor.matmul(
                                        ct[:],
                                        v_aug[:, c, h, :],
                                        es_tiles[c // 2][:, c % 2, :],
                                        start=(c == 0),
                                        stop=(c == nkc - 1),
                                    )
                                nc.vector.tensor_copy(
                                    ctxu_pair[pair][row:row + 64, qs], ct[0:HD, :]
                                )
                                nc.vector.tensor_copy(
                                    den_pair[pair][hh * 32:hh * 32 + 1, qs],
                                    ct[HD:HD + 1, :],
                                )

                        def norm_cc(pair):
                            ctxn = np_pool.tile([128, S], BF16, tag="ctxn")
                            for j in range(NQ):
                                qs = slice(j * QC, (j + 1) * QC)
                                bc = ctp.tile([128, QC], F32, tag="ct")
                                nc.tensor.matmul(
                                    bc[:], sel_sb[:], den_pair[pair][:, qs],
                                    start=True, stop=True,
                                )
                                rb = np_pool.tile([128, QC], F32, tag="rb")
                                nc.vector.reciprocal(rb[:], bc[:])
                                nc.vector.tensor_mul(
                                    ctxn[:, qs], ctxu_pair[pair][:, qs], rb[:]
                                )
                            nc.sync.dma_start(cc_in[pair][:], ctxn[:])
                            _ccs.append(nc.gpsimd.collective_compute(
                                "AllGather",
                                mybir.AluOpType.bypass,
                                replica_groups=[[0, 1, 2, 3], [4, 5, 6, 7]],
                                ins=[cc_in[pair].opt()],
                                outs=[cc_out[pair].opt()],
                            ))

                        # conservative sequential ordering: overlapped
                        # variants showed intermittent collective races
                        proj_qk(0)
                        proj_qk(1)
                        proj_v(0)
                        proj_v(1)
                        attn_head(0)
                        attn_head(1)
                        norm_cc(0)
                        attn_head(2)
                        attn_head(3)
                        norm_cc(1)

                    # ---- out-proj: outT[oc, q] = Wo[:, oc]^T ctxT + bo ----
                    # gather #p holds global ctx chunks {2r+p}; accumulate the
                    # pair-0 chunks first so they overlap gather #1
                    with tc.tile_pool(name="cpool", bufs=1) as cp, \
                         tc.tile_pool(name="out_ps", bufs=4, space="PSUM") as outp, \
                         tc.tile_pool(name="out_sb", bufs=3) as outs:
                        ctxT_sb = cp.tile([128, DCH, S], BF16, tag="ctxT_sb")
                        for p in range(2):
                            for r in range(GROUP):
                                _rds.append((nc.sync.dma_start(
                                    ctxT_sb[:, 2 * r + p, :],
                                    cc_out[p][r * 128:(r + 1) * 128, :],
                                ), p))
                        chunk_order = [2 * r for r in range(GROUP)] + \
                                      [2 * r + 1 for r in range(GROUP)]
                        for o in range(CW // 128):
                            for j in range(NQ):
                                ps = outp.tile([128, QC], F32, tag="ops")
                                for ci, c in enumerate(chunk_order):
                                    nc.tensor.matmul(
                                        ps[:],
                                        wo_sb[:, c, o * 128:(o + 1) * 128],
                                        ctxT_sb[:, c, j * QC:(j + 1) * QC],
                                        start=(ci == 0),
                                        stop=(ci == DCH - 1),
                                    )
                                ot = outs.tile([128, QC], F32, tag="ot")
                                nc.scalar.activation(
                                    ot[:], ps[:], Act.Identity, bias=bo_sb[:, o, :]
                                )
                                nc.sync.dma_start(
                                    outT[o * 128:(o + 1) * 128, j * QC:(j + 1) * QC],
                                    ot[:],
                                )

                for _rep in range(reps):
                    _emit_once()
    upd = _ccs[0].ins.sync_info.on_update[0]
    cc_done_sem = bass.SemaphoreHandle(upd.ant_name, upd.id)
    per_rep = len(_rds) // reps
    for i, (rd, p) in enumerate(_rds):
        rep = i // per_rep
        # check=False: wait slot may be taken; bacc splits into event sems
        rd.wait_op(cc_done_sem, 2 * rep + p + 1, "sem-ge", check=False)
    nc.compile()
    return nc


def _causal_mask():
    # msk[kp, r, qf] = 1 where (r*128 + kp) <= qf else 0  (keep k <= q)
    kp = np.arange(128)[:, None, None]
    r = np.arange(KPQ)[None, :, None]
    qf = np.arange(QC)[None, None, :]
    return (r * 128 + kp <= qf).astype(ml_dtypes.bfloat16)


def _in_maps(x, Wq, Wk, Wv, Wo, bo):
    bf = ml_dtypes.bfloat16
    msk = _causal_mask()
    sel33 = np.zeros((33, 128), dtype=bf)
    sel33[0, 0:64] = 1.0
    sel33[32, 64:128] = 1.0
    xT = [np.ascontiguousarray(x[b].T).astype(bf) for b in range(B)]
    maps = []
    for c in range(NCORES):
        b, g = c // GROUP, c % GROUP
        cs = slice(g * CW, (g + 1) * CW)
        maps.append({
            "xT": xT[b],
            "wq": np.ascontiguousarray(Wq[:, cs]).astype(bf),
            "wk": np.ascontiguousarray(Wk[:, cs]).astype(bf),
            "wv": np.ascontiguousarray(Wv[:, cs]).astype(bf),
            "wo": np.ascontiguousarray(Wo[:, cs]).astype(bf),
            "bo": np.ascontiguousarray(bo[cs, None]).astype(np.float32),
            "msk": msk,
            "vones": np.ones((128, NKC, HPC, 1), dtype=bf),
            "sel33": sel33,
        })
    return maps


def kernel(x, Wq, Wk, Wv, Wo, bo, _trace=False):
    x = np.asarray(x, dtype=np.float32)
    Wq, Wk, Wv, Wo, bo = (np.asarray(a, dtype=np.float32) for a in (Wq, Wk, Wv, Wo, bo))
    if "nc" not in _CACHE:
        _CACHE["nc"] = _build_bass()
    nc = _CACHE["nc"]
    res = run_bass_kernel_spmd(
        nc, _in_maps(x, Wq, Wk, Wv, Wo, bo), list(range(NCORES)), trace=_trace
    )
    out = np.zeros((B, S, D), dtype=np.float32)
    for c in range(NCORES):
        b, g = c // GROUP, c % GROUP
        out[b, :, g * CW:(g + 1) * CW] = res.results[c]["outT"].T
    if _trace:
        return out, res
    return out



# revision 4
# speedup vs baseline: 1.2579x; 1.2579x over previous
"""Distributed causal multi-head attention for Trainium2 (8 NeuronCores).

Problem (hardcoded): x[2, 2048, 1024], 16 heads, head_dim 64, causal
softmax(QK^T/8)V then out-proj with bias. f32 in/out.

Sharding: tensor parallel on heads across all 8 cores (2 heads per core),
both batches processed on every core (batch = inner loop). The ctx
exchange before the out-projection is an 8-core AllToAll per batch:
core c contributes ctx^T[128 rows = heads {2c,2c+1}, 2048 q] chunked
along q into 8 slices of 256; after the AllToAll each core holds the
full 1024-row ctx^T for ITS 256-token q-slice and computes
out[q_slice, :] = ctx^T.T @ Wo + bo with the full Wo. An AllToAll
moves 1/4 the bytes of the AllGather pair it replaces (the collective
cost is dominated by output size), and only the second one (batch 1)
sits on the critical path.

Per-core, per-batch attention (identical numerics to the AllGather
version):
  - Q^T,K^T packed 2 heads x 64 dims into 128 partitions, V per head
  - scores transposed S^T[k,q] = K Q^T so the softmax denominator comes
    out of the PE via an appended ones-column on V
  - exp without max-subtraction (scores are O(2), safe in fp32/bf16)
  - causal mask applied post-exp as a 0/1 bf16 multiply (DVE 2x mode)
  - ctx^T accumulated per q-chunk, normalized with 1/den partition-
    broadcast via a 33-row selector matmul
All matmuls bf16 (fp32 PSUM accumulation).
"""

import numpy as np
import ml_dtypes

from concourse import bass, bacc, mybir
from concourse import tile
from concourse.bass_utils import run_bass_kernel_spmd

BF16 = mybir.dt.bfloat16
F32 = mybir.dt.float32
Act = mybir.ActivationFunctionType

B, S, D = 2, 2048, 1024
H, HD = 16, 64
NCORES = 8
HPC = H // NCORES    # 2 heads per core
CW = HPC * HD        # 128 columns per core
QS = S // NCORES     # 256: per-core q-slice for the out-proj
QC = 512             # q-chunk width in attention
KC = 128             # k-chunk width
NQ = S // QC         # 4
NKC = S // KC        # 16
KPQ = QC // KC       # 4 k-chunks per q-chunk
DCH = D // 128       # 8 contraction chunks of 128
OCH = D // 128       # 8 out-proj column blocks

_CACHE = {}


def _build_bass():
    nc = bacc.Bacc(
        "TRN2", target_bir_lowering=False, debug=False, num_devices=NCORES
    )
    # Tile under-syncs readers of async collective outputs (readback DMAs can
    # fire before the exchange lands); completion waits are attached post-Tile
    _ccs = []
    _rds = []

    # per-core external inputs (same shapes on every core: SPMD)
    xT0 = nc.declare_dram_parameter("xT0", [D, S], BF16, isOutput=False)
    xT1 = nc.declare_dram_parameter("xT1", [D, S], BF16, isOutput=False)
    wq = nc.declare_dram_parameter("wq", [D, CW], BF16, isOutput=False)
    wk = nc.declare_dram_parameter("wk", [D, CW], BF16, isOutput=False)
    wv = nc.declare_dram_parameter("wv", [D, CW], BF16, isOutput=False)
    wo = nc.declare_dram_parameter("wo", [D, D], BF16, isOutput=False)
    bo = nc.declare_dram_parameter("bo", [D, 1], F32, isOutput=False)
    msk = nc.declare_dram_parameter("msk", [128, KPQ, QC], BF16, isOutput=False)
    vones = nc.declare_dram_parameter("vones", [128, NKC, HPC, 1], BF16, isOutput=False)
    # selector for den broadcast: bc[m,q] = sum_k sel33[k,m]*den_pair[k,q]
    sel33 = nc.declare_dram_parameter("sel33", [33, 128], BF16, isOutput=False)
    # rows 0-1023 batch 0, rows 1024-2047 batch 1; columns = my q-slice
    outT = nc.declare_dram_parameter("outT", [B * D, QS], F32, isOutput=True)
    xT = [xT0, xT1]

    with tile.TileContext(nc) as tc:
        with tc.tile_pool(name="dram", bufs=1, space="DRAM") as dram:
            # AllToAll buffers, one pair per batch so batch-0 comm overlaps
            # batch-1 attention. Layout [8, 128, 256]: flat chunk j is the
            # [128, 256] ctx block for q-slice j (sent to / received from
            # core j).
            cc_in = [dram.tile([NCORES, CW, QS], BF16, name=f"cc_in{b}")
                     for b in range(B)]
            cc_out = [dram.tile([NCORES, CW, QS], BF16, name=f"cc_out{b}")
                      for b in range(B)]

            with tc.tile_pool(name="persist", bufs=1) as pp:
                wq_sb = pp.tile([128, DCH, CW], BF16, tag="wq_sb")
                wk_sb = pp.tile([128, DCH, CW], BF16, tag="wk_sb")
                wv_sb = pp.tile([128, DCH, CW], BF16, tag="wv_sb")
                wo_sb = pp.tile([128, DCH, D], BF16, tag="wo_sb")
                bo_sb = pp.tile([128, OCH, 1], F32, tag="bo_sb")
                msk_sb = pp.tile([128, KPQ, QC], BF16, tag="msk_sb")
                sel_sb = pp.tile([33, 128], BF16, tag="sel_sb")
                xT_sb = [pp.tile([128, DCH, S], BF16, tag=f"xT_sb{b}", name=f"xT_sb{b}")
                         for b in range(B)]
                qT_sb = [pp.tile([128, S], BF16, tag=f"qT_sb{b}", name=f"qT_sb{b}") for b in range(B)]
                kT_sb = [pp.tile([128, S], BF16, tag=f"kT_sb{b}", name=f"kT_sb{b}") for b in range(B)]
                v_aug = [pp.tile([128, NKC, HPC, HD + 1], BF16, tag=f"v_aug{b}", name=f"v_aug{b}")
                         for b in range(B)]
                ctxu = [pp.tile([128, S], F32, tag=f"ctxu{b}", name=f"ctxu{b}") for b in range(B)]
                # den per batch: head 0 at partition 0, head 1 at partition
                # 32 (ACT writes must start at multiples of 32); rows 1-31
                # zeroed so the K=33 selector matmul can broadcast both heads
                # to output partitions 0-63 / 64-127 in one instruction
                den = [pp.tile([33, S], BF16, tag=f"den{b}", name=f"den{b}")
                       for b in range(B)]
                ctxT_sb = [pp.tile([128, DCH, QS], BF16, tag=f"ctxT_sb{b}", name=f"ctxT_sb{b}")
                           for b in range(B)]
                for b in range(B):
                    nc.vector.memset(den[b][:], 0.0)

                # DMA order matters for startup latency: wq + x(b0) first so
                # the projection matmuls can start streaming, wo/bo last
                for c in range(DCH):
                    nc.sync.dma_start(wq_sb[:, c, :], wq[c * 128:(c + 1) * 128, :])
                for c in range(DCH):
                    nc.sync.dma_start(xT_sb[0][:, c, :], xT0[c * 128:(c + 1) * 128, :])
                for c in range(DCH):
                    nc.sync.dma_start(wk_sb[:, c, :], wk[c * 128:(c + 1) * 128, :])
                for c in range(DCH):
                    nc.sync.dma_start(wv_sb[:, c, :], wv[c * 128:(c + 1) * 128, :])
                nc.sync.dma_start(msk_sb[:], msk[:])
                for b in range(B):
                    nc.sync.dma_start(v_aug[b][:, :, :, HD:HD + 1], vones[:])
                nc.sync.dma_start(sel_sb[:], sel33[:])
                for c in range(DCH):
                    nc.sync.dma_start(xT_sb[1][:, c, :], xT1[c * 128:(c + 1) * 128, :])
                for c in range(DCH):
                    nc.sync.dma_start(wo_sb[:, c, :], wo[c * 128:(c + 1) * 128, :])
                for o in range(OCH):
                    nc.sync.dma_start(bo_sb[:, o, :], bo[o * 128:(o + 1) * 128, :])

                # All PSUM pools coexist (phases interleave): 2+4+2 banks.
                # mm_ps is shared by the projections and the out-proj (they
                # never contend: proj(b1) overlaps attn(b0), outproj(b0)
                # overlaps attn(b1)).
                with tc.tile_pool(name="mm_ps", bufs=2, space="PSUM") as mmp, \
                     tc.tile_pool(name="sc_ps", bufs=2, space="PSUM") as scp, \
                     tc.tile_pool(name="ctbc_ps", bufs=2, space="PSUM") as ctp, \
                     tc.tile_pool(name="es_pool", bufs=NKC // 2 + 2) as esp, \
                     tc.tile_pool(name="norm", bufs=2) as np_pool, \
                     tc.tile_pool(name="out_sb", bufs=3) as outs:

                    def proj_qk(b):
                        for w_sb, dst in ((wq_sb, qT_sb[b]), (wk_sb, kT_sb[b])):
                            for j in range(NQ):
                                ps = mmp.tile([128, QC], F32, tag="mm")
                                for c in range(DCH):
                                    nc.tensor.matmul(
                                        ps[:],
                                        w_sb[:, c, :],
                                        xT_sb[b][:, c, j * QC:(j + 1) * QC],
                                        start=(c == 0),
                                        stop=(c == DCH - 1),
                                    )
                                nc.vector.tensor_copy(
                                    dst[:, j * QC:(j + 1) * QC], ps[:]
                                )

                    def proj_v(b):
                        # V for this core's 2 heads: [tok, 2*64]
                        for t in range(NKC):
                            ps = mmp.tile([128, QC], F32, tag="mm")
                            for c in range(DCH):
                                nc.tensor.matmul(
                                    ps[:, 0:128],
                                    xT_sb[b][:, c, t * 128:(t + 1) * 128],
                                    wv_sb[:, c, :],
                                    start=(c == 0),
                                    stop=(c == DCH - 1),
                                )
                            nc.vector.tensor_copy(
                                v_aug[b][:, t, :, 0:HD],
                                ps[:, 0:128].rearrange("p (h w) -> p h w", h=HPC),
                            )

                    def attn_head(b, h):
                        row = h * HD
                        for j in range(NQ):
                            nkc = (j + 1) * KPQ
                            qs = slice(j * QC, (j + 1) * QC)
                            es_tiles = []
                            for c0 in range(0, nkc, 2):
                                # two k-chunks share one 2-bank PSUM tile
                                # -> one exp instruction
                                st = scp.tile([128, 2, QC], F32, tag="st")
                                for i in range(2):
                                    c = c0 + i
                                    nc.tensor.matmul(
                                        st[:, i, :],
                                        kT_sb[b][row:row + HD, c * KC:(c + 1) * KC],
                                        qT_sb[b][row:row + HD, qs],
                                        start=True, stop=True,
                                    )
                                es = esp.tile([128, 2, QC], BF16, tag="es")
                                nc.scalar.activation(es[:], st[:], Act.Exp, scale=0.125)
                                if c0 >= j * KPQ:
                                    r = c0 - j * KPQ
                                    nc.vector.tensor_mul(
                                        es[:], es[:], msk_sb[:, r:r + 2, :]
                                    )
                                es_tiles.append(es)
                            ct = ctp.tile([HD + 1, QC], F32, tag="ct")
                            for c in range(nkc):
                                nc.tensor.matmul(
                                    ct[:],
                                    v_aug[b][:, c, h, :],
                                    es_tiles[c // 2][:, c % 2, :],
                                    start=(c == 0),
                                    stop=(c == nkc - 1),
                                )
                            nc.vector.tensor_copy(
                                ctxu[b][row:row + HD, qs], ct[0:HD, :]
                            )
                            nc.vector.tensor_copy(
                                den[b][h * 32:h * 32 + 1, qs],
                                ct[HD:HD + 1, :],
                            )

                    def norm_cc(b):
                        ctxn = np_pool.tile([128, S], BF16, tag="ctxn")
                        for j in range(NQ):
                            qs = slice(j * QC, (j + 1) * QC)
                            bc = ctp.tile([128, QC], F32, tag="ct")
                            nc.tensor.matmul(
                                bc[:], sel_sb[:], den[b][:, qs],
                                start=True, stop=True,
                            )
                            rb = np_pool.tile([128, QC], F32, tag="rb")
                            nc.vector.reciprocal(rb[:], bc[:])
                            nc.vector.tensor_mul(
                                ctxn[:, qs], ctxu[b][:, qs], rb[:]
                            )
                        # chunk the 2048 q-columns into 8 blocks of 256 so the
                        # AllToAll's flat chunk j is q-slice j
                        for j in range(NCORES):
                            nc.sync.dma_start(
                                cc_in[b][j, :, :],
                                ctxn[:, j * QS:(j + 1) * QS],
                            )
                        _ccs.append(nc.gpsimd.collective_compute(
                            "AllToAll",
                            mybir.AluOpType.bypass,
                            replica_groups=[list(range(NCORES))],
                            ins=[cc_in[b].opt()],
                            outs=[cc_out[b].opt()],
                        ))

                    def readback(b):
                        for c in range(DCH):
                            _rds.append((nc.sync.dma_start(
                                ctxT_sb[b][:, c, :],
                                cc_out[b][c, :, :],
                            ), b))

                    def out_proj(b):
                        # outT[oc, q_slice] = Wo[:, oc]^T ctxT + bo[oc]
                        for o in range(OCH):
                            ps = mmp.tile([128, QC], F32, tag="mm")
                            for c in range(DCH):
                                nc.tensor.matmul(
                                    ps[:, 0:QS],
                                    wo_sb[:, c, o * 128:(o + 1) * 128],
                                    ctxT_sb[b][:, c, :],
                                    start=(c == 0),
                                    stop=(c == DCH - 1),
                                )
                            ot = outs.tile([128, QS], F32, tag="ot")
                            nc.scalar.activation(
                                ot[:], ps[:, 0:QS], Act.Identity, bias=bo_sb[:, o, :]
                            )
                            nc.sync.dma_start(
                                outT[b * D + o * 128:b * D + (o + 1) * 128, :],
                                ot[:],
                            )

                    proj_qk(0)
                    proj_v(0)
                    attn_head(0, 0)
                    proj_qk(1)
                    attn_head(0, 1)
                    proj_v(1)
                    norm_cc(0)
                    readback(0)
                    attn_head(1, 0)
                    out_proj(0)
                    attn_head(1, 1)
                    norm_cc(1)
                    readback(1)
                    out_proj(1)

    # attach completion waits: readback DMAs for batch b must observe the
    # b-th collective's completion semaphore
    upd0 = _ccs[0].ins.sync_info.on_update[0]
    upd1 = _ccs[1].ins.sync_info.on_update[0]
    assert (upd0.ant_name, upd0.id) == (upd1.ant_name, upd1.id), (
        "collectives use distinct sems; adjust wait thresholds"
    )
    cc_done_sem = bass.SemaphoreHandle(upd0.ant_name, upd0.id)
    for rd, b in _rds:
        # check=False: wait slot may be taken; bacc splits into event sems
        rd.wait_op(cc_done_sem, b + 1, "sem-ge", check=False)
    nc.compile()
    return nc


def _causal_mask():
    # msk[kp, r, qf] = 1 where (r*128 + kp) <= qf else 0  (keep k <= q)
    kp = np.arange(128)[:, None, None]
    r = np.arange(KPQ)[None, :, None]
    qf = np.arange(QC)[None, None, :]
    return (r * 128 + kp <= qf).astype(ml_dtypes.bfloat16)


def _in_maps(x, Wq, Wk, Wv, Wo, bo):
    bf = ml_dtypes.bfloat16
    msk = _causal_mask()
    sel33 = np.zeros((33, 128), dtype=bf)
    sel33[0, 0:64] = 1.0
    sel33[32, 64:128] = 1.0
    xT = [np.ascontiguousarray(x[b].T).astype(bf) for b in range(B)]
    wo_full = np.ascontiguousarray(Wo).astype(bf)
    bo_full = np.ascontiguousarray(bo[:, None]).astype(np.float32)
    vones = np.ones((128, NKC, HPC, 1), dtype=bf)
    maps = []
    for c in range(NCORES):
        cs = slice(c * CW, (c + 1) * CW)
        maps.append({
            "xT0": xT[0],
            "xT1": xT[1],
            "wq": np.ascontiguousarray(Wq[:, cs]).astype(bf),
            "wk": np.ascontiguousarray(Wk[:, cs]).astype(bf),
            "wv": np.ascontiguousarray(Wv[:, cs]).astype(bf),
            "wo": wo_full,
            "bo": bo_full,
            "msk": msk,
            "vones": vones,
            "sel33": sel33,
        })
    return maps


def kernel(x, Wq, Wk, Wv, Wo, bo, _trace=False):
    x = np.asarray(x, dtype=np.float32)
    Wq, Wk, Wv, Wo, bo = (np.asarray(a, dtype=np.float32) for a in (Wq, Wk, Wv, Wo, bo))
    if "nc" not in _CACHE:
        _CACHE["nc"] = _build_bass()
    nc = _CACHE["nc"]
    res = run_bass_kernel_spmd(
        nc, _in_maps(x, Wq, Wk, Wv, Wo, bo), list(range(NCORES)), trace=_trace
    )
    out = np.zeros((B, S, D), dtype=np.float32)
    for c in range(NCORES):
        qs = slice(c * QS, (c + 1) * QS)
        for b in range(B):
            out[b, qs, :] = res.results[c]["outT"][b * D:(b + 1) * D, :].T
    if _trace:
        return out, res
    return out


# revision 7
# speedup vs baseline: 1.2589x; 1.0008x over previous
"""Distributed causal multi-head attention for Trainium2 (8 NeuronCores).

Problem (hardcoded): x[2, 2048, 1024], 16 heads, head_dim 64, causal
softmax(QK^T/8)V then out-proj with bias. f32 in/out.

Sharding: tensor parallel on heads across all 8 cores (2 heads per core),
both batches processed on every core (batch = inner loop). The ctx
exchange before the out-projection is an 8-core AllToAll per batch:
core c contributes ctx^T[128 rows = heads {2c,2c+1}, 2048 q] chunked
along q into 8 slices of 256; after the AllToAll each core holds the
full 1024-row ctx^T for ITS 256-token q-slice and computes
out[q_slice, :] = ctx^T.T @ Wo + bo with the full Wo. An AllToAll
moves 1/4 the bytes of the AllGather pair it replaces (the collective
cost is dominated by output size), and only the second one (batch 1)
sits on the critical path.

Per-core, per-batch attention (identical numerics to the AllGather
version):
  - Q^T,K^T packed 2 heads x 64 dims into 128 partitions, V per head
  - scores transposed S^T[k,q] = K Q^T so the softmax denominator comes
    out of the PE via an appended ones-column on V
  - exp without max-subtraction (scores are O(2), safe in fp32/bf16)
  - causal mask applied post-exp as a 0/1 bf16 multiply (DVE 2x mode)
  - ctx^T accumulated per q-chunk, normalized with 1/den partition-
    broadcast via a 33-row selector matmul
All matmuls bf16 (fp32 PSUM accumulation).
"""

import numpy as np
import ml_dtypes

from concourse import bass, bacc, mybir
from concourse import tile
from concourse.bass_utils import run_bass_kernel_spmd

BF16 = mybir.dt.bfloat16
F32 = mybir.dt.float32
Act = mybir.ActivationFunctionType

B, S, D = 2, 2048, 1024
H, HD = 16, 64
NCORES = 8
HPC = H // NCORES    # 2 heads per core
CW = HPC * HD        # 128 columns per core
QS = S // NCORES     # 256: per-core q-slice for the out-proj
QC = 512             # q-chunk width in attention
KC = 128             # k-chunk width
NQ = S // QC         # 4
NKC = S // KC        # 16
KPQ = QC // KC       # 4 k-chunks per q-chunk
DCH = D // 128       # 8 contraction chunks of 128
OCH = D // 128       # 8 out-proj column blocks

_CACHE = {}


def _build_bass():
    nc = bacc.Bacc(
        "TRN2", target_bir_lowering=False, debug=False, num_devices=NCORES
    )
    # Tile under-syncs readers of async collective outputs (readback DMAs can
    # fire before the exchange lands); completion waits are attached post-Tile
    _ccs = []
    _rds = []

    # per-core external inputs (same shapes on every core: SPMD)
    xT0 = nc.declare_dram_parameter("xT0", [D, S], BF16, isOutput=False)
    xT1 = nc.declare_dram_parameter("xT1", [D, S], BF16, isOutput=False)
    wq = nc.declare_dram_parameter("wq", [D, CW], BF16, isOutput=False)
    wk = nc.declare_dram_parameter("wk", [D, CW], BF16, isOutput=False)
    wv = nc.declare_dram_parameter("wv", [D, CW], BF16, isOutput=False)
    wo = nc.declare_dram_parameter("wo", [D, D], BF16, isOutput=False)
    bo = nc.declare_dram_parameter("bo", [D, 1], F32, isOutput=False)
    msk = nc.declare_dram_parameter("msk", [128, KPQ, QC], BF16, isOutput=False)
    vones = nc.declare_dram_parameter("vones", [128, NKC, HPC, 1], BF16, isOutput=False)
    # selector for den broadcast: bc[m,q] = sum_k sel33[k,m]*den_pair[k,q]
    sel33 = nc.declare_dram_parameter("sel33", [33, 128], BF16, isOutput=False)
    # rows 0-1023 batch 0, rows 1024-2047 batch 1; columns = my q-slice
    outT = nc.declare_dram_parameter("outT", [B * D, QS], F32, isOutput=True)
    xT = [xT0, xT1]

    with tile.TileContext(nc) as tc:
        with tc.tile_pool(name="dram", bufs=1, space="DRAM") as dram:
            # AllToAll buffers, one pair per batch so batch-0 comm overlaps
            # batch-1 attention. Layout [8, 128, 256]: flat chunk j is the
            # [128, 256] ctx block for q-slice j (sent to / received from
            # core j).
            cc_in = [dram.tile([NCORES, CW, QS], BF16, name=f"cc_in{b}")
                     for b in range(B)]
            cc_out = [dram.tile([NCORES, CW, QS], BF16, name=f"cc_out{b}")
                      for b in range(B)]

            with tc.tile_pool(name="persist", bufs=1) as pp:
                wq_sb = pp.tile([128, DCH, CW], BF16, tag="wq_sb")
                wk_sb = pp.tile([128, DCH, CW], BF16, tag="wk_sb")
                wv_sb = pp.tile([128, DCH, CW], BF16, tag="wv_sb")
                wo_sb = pp.tile([128, DCH, D], BF16, tag="wo_sb")
                bo_sb = pp.tile([128, OCH, 1], F32, tag="bo_sb")
                msk_sb = pp.tile([128, KPQ, QC], BF16, tag="msk_sb")
                sel_sb = pp.tile([33, 128], BF16, tag="sel_sb")
                xT_sb = [pp.tile([128, DCH, S], BF16, tag=f"xT_sb{b}", name=f"xT_sb{b}")
                         for b in range(B)]
                qT_sb = [pp.tile([128, S], BF16, tag=f"qT_sb{b}", name=f"qT_sb{b}") for b in range(B)]
                kT_sb = [pp.tile([128, S], BF16, tag=f"kT_sb{b}", name=f"kT_sb{b}") for b in range(B)]
                v_aug = [pp.tile([128, NKC, HPC, HD + 1], BF16, tag=f"v_aug{b}", name=f"v_aug{b}")
                         for b in range(B)]
                ctxu = [pp.tile([128, S], F32, tag=f"ctxu{b}", name=f"ctxu{b}") for b in range(B)]
                # den per batch: head 0 at partition 0, head 1 at partition
                # 32 (ACT writes must start at multiples of 32); rows 1-31
                # zeroed so the K=33 selector matmul can broadcast both heads
                # to output partitions 0-63 / 64-127 in one instruction
                den = [pp.tile([33, S], BF16, tag=f"den{b}", name=f"den{b}")
                       for b in range(B)]
                ctxT_sb = [pp.tile([128, DCH, QS], BF16, tag=f"ctxT_sb{b}", name=f"ctxT_sb{b}")
                           for b in range(B)]
                for b in range(B):
                    nc.vector.memset(den[b][:], 0.0)

                # DMA order matters for startup latency: wq + x(b0) first so
                # the projection matmuls can start streaming, wo/bo last
                nc.sync.dma_start(wq_sb[:], wq.rearrange("(c p) w -> p c w", p=128))
                for c in range(DCH):
                    nc.sync.dma_start(xT_sb[0][:, c, :], xT0[c * 128:(c + 1) * 128, :])
                nc.sync.dma_start(wk_sb[:], wk.rearrange("(c p) w -> p c w", p=128))
                nc.sync.dma_start(wv_sb[:], wv.rearrange("(c p) w -> p c w", p=128))
                nc.sync.dma_start(msk_sb[:], msk[:])
                for b in range(B):
                    nc.sync.dma_start(v_aug[b][:, :, :, HD:HD + 1], vones[:])
                nc.sync.dma_start(sel_sb[:], sel33[:])
                for c in range(DCH):
                    nc.sync.dma_start(xT_sb[1][:, c, :], xT1[c * 128:(c + 1) * 128, :])
                nc.sync.dma_start(wo_sb[:], wo.rearrange("(c p) w -> p c w", p=128))
                nc.sync.dma_start(bo_sb[:], bo.rearrange("(o p) z -> p o z", p=128))

                # All PSUM pools coexist (phases interleave): 2+4+2 banks.
                # mm_ps is shared by the projections and the out-proj (they
                # never contend: proj(b1) overlaps attn(b0), outproj(b0)
                # overlaps attn(b1)).
                with tc.tile_pool(name="mm_ps", bufs=2, space="PSUM") as mmp, \
                     tc.tile_pool(name="sc_ps", bufs=2, space="PSUM") as scp, \
                     tc.tile_pool(name="ctbc_ps", bufs=2, space="PSUM") as ctp, \
                     tc.tile_pool(name="es_pool", bufs=NKC // 2 + 2) as esp, \
                     tc.tile_pool(name="norm", bufs=2) as np_pool, \
                     tc.tile_pool(name="out_sb", bufs=3) as outs:

                    def proj_qk(b):
                        for w_sb, dst in ((wq_sb, qT_sb[b]), (wk_sb, kT_sb[b])):
                            for j in range(NQ):
                                ps = mmp.tile([128, QC], F32, tag="mm")
                                for c in range(DCH):
                                    nc.tensor.matmul(
                                        ps[:],
                                        w_sb[:, c, :],
                                        xT_sb[b][:, c, j * QC:(j + 1) * QC],
                                        start=(c == 0),
                                        stop=(c == DCH - 1),
                                    )
                                nc.vector.tensor_copy(
                                    dst[:, j * QC:(j + 1) * QC], ps[:]
                                )

                    def proj_v(b):
                        # V for this core's 2 heads: [tok, 2*64]
                        for t in range(NKC):
                            ps = mmp.tile([128, QC], F32, tag="mm")
                            for c in range(DCH):
                                nc.tensor.matmul(
                                    ps[:, 0:128],
                                    xT_sb[b][:, c, t * 128:(t + 1) * 128],
                                    wv_sb[:, c, :],
                                    start=(c == 0),
                                    stop=(c == DCH - 1),
                                )
                            nc.vector.tensor_copy(
                                v_aug[b][:, t, :, 0:HD],
                                ps[:, 0:128].rearrange("p (h w) -> p h w", h=HPC),
                            )

                    def attn_head(b, h):
                        row = h * HD
                        for j in range(NQ):
                            nkc = (j + 1) * KPQ
                            qs = slice(j * QC, (j + 1) * QC)
                            es_tiles = []
                            for c0 in range(0, nkc, 2):
                                # two k-chunks share one 2-bank PSUM tile
                                # -> one exp instruction
                                st = scp.tile([128, 2, QC], F32, tag="st")
                                for i in range(2):
                                    c = c0 + i
                                    nc.tensor.matmul(
                                        st[:, i, :],
                                        kT_sb[b][row:row + HD, c * KC:(c + 1) * KC],
                                        qT_sb[b][row:row + HD, qs],
                                        start=True, stop=True,
                                    )
                                es = esp.tile([128, 2, QC], BF16, tag="es")
                                nc.scalar.activation(es[:], st[:], Act.Exp, scale=0.125)
                                if c0 >= j * KPQ:
                                    r = c0 - j * KPQ
                                    nc.vector.tensor_mul(
                                        es[:], es[:], msk_sb[:, r:r + 2, :]
                                    )
                                es_tiles.append(es)
                            ct = ctp.tile([HD + 1, QC], F32, tag="ct")
                            for c in range(nkc):
                                nc.tensor.matmul(
                                    ct[:],
                                    v_aug[b][:, c, h, :],
                                    es_tiles[c // 2][:, c % 2, :],
                                    start=(c == 0),
                                    stop=(c == nkc - 1),
                                )
                            nc.vector.tensor_copy(
                                ctxu[b][row:row + HD, qs], ct[0:HD, :]
                            )
                            nc.vector.tensor_copy(
                                den[b][h * 32:h * 32 + 1, qs],
                                ct[HD:HD + 1, :],
                            )

                    def norm_cc(b):
                        ctxn = np_pool.tile([128, S], BF16, tag="ctxn")
                        for j in range(NQ):
                            qs = slice(j * QC, (j + 1) * QC)
                            bc = ctp.tile([128, QC], F32, tag="ct")
                            nc.tensor.matmul(
                                bc[:], sel_sb[:], den[b][:, qs],
                                start=True, stop=True,
                            )
                            rb = np_pool.tile([128, QC], F32, tag="rb")
                            nc.vector.reciprocal(rb[:], bc[:])
                            nc.vector.tensor_mul(
                                ctxn[:, qs], ctxu[b][:, qs], rb[:]
                            )
                        # chunk the 2048 q-columns into 8 blocks of 256 so the
                        # AllToAll's flat chunk j is q-slice j (single DMA:
                        # both sides viewed partition-first [128, 8, 256])
                        nc.sync.dma_start(
                            cc_in[b].rearrange("c p q -> p c q"),
                            ctxn.rearrange("p (c q) -> p c q", c=NCORES),
                        )
                        _ccs.append(nc.gpsimd.collective_compute(
                            "AllToAll",
                            mybir.AluOpType.bypass,
                            replica_groups=[list(range(NCORES))],
                            ins=[cc_in[b].opt()],
                            outs=[cc_out[b].opt()],
                        ))

                    def readback(b):
                        _rds.append((nc.sync.dma_start(
                            ctxT_sb[b][:, :, :],
                            cc_out[b].rearrange("c p q -> p c q"),
                        ), b))

                    def out_proj(b):
                        # outT[oc, q_slice] = Wo[:, oc]^T ctxT + bo[oc]
                        for o in range(OCH):
                            ps = mmp.tile([128, QC], F32, tag="mm")
                            for c in range(DCH):
                                nc.tensor.matmul(
                                    ps[:, 0:QS],
                                    wo_sb[:, c, o * 128:(o + 1) * 128],
                                    ctxT_sb[b][:, c, :],
                                    start=(c == 0),
                                    stop=(c == DCH - 1),
                                )
                            ot = outs.tile([128, QS], F32, tag="ot")
                            nc.scalar.activation(
                                ot[:], ps[:, 0:QS], Act.Identity, bias=bo_sb[:, o, :]
                            )
                            nc.sync.dma_start(
                                outT[b * D + o * 128:b * D + (o + 1) * 128, :],
                                ot[:],
                            )

                    proj_qk(0)
                    proj_v(0)
                    attn_head(0, 0)
                    proj_qk(1)
                    attn_head(0, 1)
                    proj_v(1)
                    norm_cc(0)
                    readback(0)
                    attn_head(1, 0)
                    out_proj(0)
                    attn_head(1, 1)
                    norm_cc(1)
                    readback(1)
                    out_proj(1)

    # attach completion waits: readback DMAs for batch b must observe the
    # b-th collective's completion semaphore
    upd0 = _ccs[0].ins.sync_info.on_update[0]
    upd1 = _ccs[1].ins.sync_info.on_update[0]
    assert (upd0.ant_name, upd0.id) == (upd1.ant_name, upd1.id), (
        "collectives use distinct sems; adjust wait thresholds"
    )
    cc_done_sem = bass.SemaphoreHandle(upd0.ant_name, upd0.id)
    for rd, b in _rds:
        # check=False: wait slot may be taken; bacc splits into event sems
        rd.wait_op(cc_done_sem, b + 1, "sem-ge", check=False)
    nc.compile()
    return nc


def _causal_mask():
    # msk[kp, r, qf] = 1 where (r*128 + kp) <= qf else 0  (keep k <= q)
    kp = np.arange(128)[:, None, None]
    r = np.arange(KPQ)[None, :, None]
    qf = np.arange(QC)[None, None, :]
    return (r * 128 + kp <= qf).astype(ml_dtypes.bfloat16)


def _in_maps(x, Wq, Wk, Wv, Wo, bo):
    bf = ml_dtypes.bfloat16
    msk = _causal_mask()
    sel33 = np.zeros((33, 128), dtype=bf)
    sel33[0, 0:64] = 1.0
    sel33[32, 64:128] = 1.0
    xT = [np.ascontiguousarray(x[b].T).astype(bf) for b in range(B)]
    wo_full = np.ascontiguousarray(Wo).astype(bf)
    bo_full = np.ascontiguousarray(bo[:, None]).astype(np.float32)
    vones = np.ones((128, NKC, HPC, 1), dtype=bf)
    maps = []
    for c in range(NCORES):
        cs = slice(c * CW, (c + 1) * CW)
        maps.append({
            "xT0": xT[0],
            "xT1": xT[1],
            "wq": np.ascontiguousarray(Wq[:, cs]).astype(bf),
            "wk": np.ascontiguousarray(Wk[:, cs]).astype(bf),
            "wv": np.ascontiguousarray(Wv[:, cs]).astype(bf),
            "wo": wo_full,
            "bo": bo_full,
            "msk": msk,
            "vones": vones,
            "sel33": sel33,
        })
    return maps


def kernel(x, Wq, Wk, Wv, Wo, bo, _trace=False):
    x = np.asarray(x, dtype=np.float32)
    Wq, Wk, Wv, Wo, bo = (np.asarray(a, dtype=np.float32) for a in (Wq, Wk, Wv, Wo, bo))
    if "nc" not in _CACHE:
        _CACHE["nc"] = _build_bass()
    nc = _CACHE["nc"]
    res = run_bass_kernel_spmd(
        nc, _in_maps(x, Wq, Wk, Wv, Wo, bo), list(range(NCORES)), trace=_trace
    )
    out = np.zeros((B, S, D), dtype=np.float32)
    for c in range(NCORES):
        qs = slice(c * QS, (c + 1) * QS)
        for b in range(B):
            out[b, qs, :] = res.results[c]["outT"][b * D:(b + 1) * D, :].T
    if _trace:
        return out, res
    return out
